# revision 1
# baseline (speedup 1.0000x reference)
"""Sequence-parallel causal attention for Trainium2, 8 NeuronCores (SPMD).

o = softmax(causal((q@w_q)(k@w_k)^T/sqrt(dk))) @ (v@w_v) @ w_o
q/k/v [8192,1024] f32, w_q/w_k/w_v [1024,128], w_o [128,1024].

Design (v2):
- 32 superblocks (sb) of 256 rows. Core c owns sbs {c, 15-c, 16+c, 31-c},
  one per "band" of 8 sbs; band so is processed against keys
  [0, 2048*(so+1)) — uniform instruction stream, causal boundary always in
  the last 2048-key "window" (16 tiles), handled by per-core mask strips W.
- Keys contiguously sharded; projections distributed; kp (f32) and vp (fp16)
  exchanged via one merged AllGather.
- QK and stats in f32r single-product matmuls (1 cyc/row at free>=256),
  projections in fp16 hi/lo multi-product (near-f32), AV in fp16.
- Per band: body stats (unmasked, s-layout, DVE/Pool reduce) + window sT
  QK tiles masked by W, parked in SBUF f32; window row-max extracted via
  elementwise max + PE transpose; exp with exact per-row max folded in
  (rank-1 matmul for body tiles, broadcast add for window tiles).
"""

import math
import numpy as np

N_CORES = 8
S, D, DK = 8192, 1024, 128
RSB = 256                    # rows per superblock
NSB = S // RSB               # 32 superblocks
NBAND = 4                    # bands (sbs per core)
NEG_BIG = -2.0e9
INV_SQRT_DK = 1.0 / math.sqrt(DK)

_CACHE = {}


def _sb_list(c):
    return [c, 15 - c, 16 + c, 31 - c]


def _build():
    import concourse.mybir as mybir
    import concourse.tile as tile
    from concourse import bacc
    from contextlib import ExitStack

    dt = mybir.dt
    f32, f16, f32r = dt.float32, dt.float16, dt.float32r
    i32 = dt.int32

    nc = bacc.Bacc("TRN2", target_bir_lowering=False, debug=False,
                   num_devices=N_CORES)

    # ---- I/O ----  (all transposed/split on host)
    qT_h = nc.declare_dram_parameter("qT_h", [D, 1024], f16, isOutput=False)
    qT_l = nc.declare_dram_parameter("qT_l", [D, 1024], f16, isOutput=False)
    kT_h = nc.declare_dram_parameter("kT_h", [D, 1024], f16, isOutput=False)
    kT_l = nc.declare_dram_parameter("kT_l", [D, 1024], f16, isOutput=False)
    vT_h = nc.declare_dram_parameter("vT_h", [D, 1024], f16, isOutput=False)
    wq_h = nc.declare_dram_parameter("wq_h", [D, DK], f16, isOutput=False)
    wq_l = nc.declare_dram_parameter("wq_l", [D, DK], f16, isOutput=False)
    wk_h = nc.declare_dram_parameter("wk_h", [D, DK], f16, isOutput=False)
    wk_l = nc.declare_dram_parameter("wk_l", [D, DK], f16, isOutput=False)
    wv_h = nc.declare_dram_parameter("wv_h", [D, DK], f16, isOutput=False)
    wv_l = nc.declare_dram_parameter("wv_l", [D, DK], f16, isOutput=False)
    wo_p = nc.declare_dram_parameter("wo", [DK, D], f32r, isOutput=False)
    # W masks: [128 kk, 2 parity * 16 tiles * 256 rr] f32
    wmask_p = nc.declare_dram_parameter("wmask", [128, 2 * 16 * 256], f32,
                                        isOutput=False)
    ident_p = nc.declare_dram_parameter("ident", [128, 128], f32, isOutput=False)
    nones_p = nc.declare_dram_parameter("negones", [1, 128], f32r, isOutput=False)
    ones_p = nc.declare_dram_parameter("ones_col", [128, 2], f16, isOutput=False)
    o_sh = nc.declare_dram_parameter("o_sh", [1024, D], f32, isOutput=True)

    # ---- internal DRAM for the merged AllGather ----
    # int32 so the collective treats payload as raw bytes (fp16 pairs would
    # hit float canonicalization/FTZ in the fp32 datapath)
    # [128, 1024 kp-f32-bits + 512 vp-fp16-pair-bits] per core
    agg_in = nc.dram_tensor("agg_in", [128, 1536], i32)
    agg_out = nc.dram_tensor("agg_out", [N_CORES, 128, 1536], i32,
                             addr_space="Shared")
    rgroups = [list(range(N_CORES))]

    with tile.TileContext(nc) as tc, ExitStack() as ctx:
        consts = ctx.enter_context(tc.tile_pool(name="consts", bufs=1))
        persist = ctx.enter_context(tc.tile_pool(name="persist", bufs=1))

        ident = consts.tile([128, 128], f32, tag="ident")
        nc.sync.dma_start(ident[:], ident_p[:])
        negones = consts.tile([1, 128], f32r, tag="negones")
        nc.sync.dma_start(negones[:], nones_p[:])
        ones_col = consts.tile([128, 2], f16, tag="ones")
        nc.sync.dma_start(ones_col[:], ones_p[:])
        wo_sb = consts.tile([128, D], f32r, tag="wo")
        nc.sync.dma_start(wo_sb[:], wo_p[:])
        wmask = consts.tile([128, 2 * 16 * 256], f32, tag="wmask")
        nc.sync.dma_start(wmask[:], wmask_p[:])

        def load_w(name, param):
            t = consts.tile([128, 8 * DK], f16, tag=name, name=name)
            nc.sync.dma_start(
                t[:].rearrange("p (di dk) -> p di dk", di=8),
                param[:].rearrange("(di p) dk -> p di dk", p=128))
            return t
        wqh_sb = load_w("wqh", wq_h); wql_sb = load_w("wql", wq_l)
        wkh_sb = load_w("wkh", wk_h); wkl_sb = load_w("wkl", wk_l)
        wvh_sb = load_w("wvh", wv_h); wvl_sb = load_w("wvl", wv_l)

        # persistent activation tiles
        qpT = persist.tile([128, 1024], f32r, tag="qpT")
        kpT = persist.tile([128, S], f32r, tag="kpT")
        vp_sb = persist.tile([128, S], f16, tag="vp")

        def load_xT(xtp, name, ph, pl):
            th = xtp.tile([128, 8 * 1024], f16, tag="xh", name=name + "h")
            nc.sync.dma_start(
                th[:].rearrange("p (di r) -> p di r", di=8),
                ph[:].rearrange("(di p) r -> p di r", p=128))
            tl = None
            if pl is not None:
                tl = xtp.tile([128, 8 * 1024], f16, tag="xl",
                              name=name + "l")
                nc.sync.dma_start(
                    tl[:].rearrange("p (di r) -> p di r", di=8),
                    pl[:].rearrange("(di p) r -> p di r", p=128))
            return th, tl

        # ---------- phase A: projections + merged AllGather ----------
        with tc.tile_pool(name="projps", bufs=2, space="PSUM") as projps, \
             tc.tile_pool(name="vps", bufs=2, space="PSUM") as vps, \
             tc.tile_pool(name="xtp", bufs=2) as xtp, \
             tc.tile_pool(name="projsb", bufs=1) as projsb:

            # ---- k ----
            kTh, kTl = load_xT(xtp, "kT", kT_h, kT_l)
            kp_own = projsb.tile([128, 1024], f32r, tag="kp_own", name="kp_own")
            for half in range(2):
                ps = projps.tile([128, 512], f32, tag="projps")
                sl = slice(512 * half, 512 * (half + 1))
                n = 0
                for wt, xt in ((wkh_sb, kTh), (wkl_sb, kTh), (wkh_sb, kTl)):
                    for di in range(8):
                        nc.tensor.matmul(
                            ps[:], wt[:, 128 * di:128 * (di + 1)],
                            xt[:, 1024 * di + 512 * half:
                               1024 * di + 512 * (half + 1)],
                            start=(n == 0), stop=(n == 23))
                        n += 1
                nc.scalar.copy(kp_own[:, sl], ps[:])
            nc.sync.dma_start(agg_in[:, :1024], kp_own[:].bitcast(i32))

            # ---- v ----  vp tiles [keys 128, dv 128] fp16
            vTh, _ = load_xT(xtp, "vT", vT_h, None)
            vp_own = projsb.tile([128, 1024], f16, tag="vp_own", name="vp_own")
            for kt in range(8):
                ps = vps.tile([128, 128], f32, tag="vps")
                n = 0
                for wt in (wvh_sb, wvl_sb):
                    for di in range(8):
                        nc.tensor.matmul(
                            ps[:],
                            vTh[:, 1024 * di + 128 * kt:1024 * di + 128 * (kt + 1)],
                            wt[:, 128 * di:128 * (di + 1)],
                            start=(n == 0), stop=(n == 15))
                        n += 1
                nc.scalar.copy(vp_own[:, 128 * kt:128 * (kt + 1)], ps[:])
            vp_pack = projsb.tile([128, 512], i32, tag="vp_pack",
                                  name="vp_pack")
            nc.vector.tensor_copy(vp_pack[:].bitcast(f16), vp_own[:])
            nc.sync.dma_start(agg_in[:, 1024:], vp_pack[:])

            nc.gpsimd.collective_compute(
                "AllGather", mybir.AluOpType.bypass, replica_groups=rgroups,
                ins=[agg_in[:]], outs=[agg_out[:]])

            # ---- q ---- (overlaps the collective)
            qTh, qTl = load_xT(xtp, "qT", qT_h, qT_l)
            for half in range(2):
                ps = projps.tile([128, 512], f32, tag="projps")
                sl = slice(512 * half, 512 * (half + 1))
                n = 0
                for wt, xt in ((wqh_sb, qTh), (wql_sb, qTh), (wqh_sb, qTl)):
                    for di in range(8):
                        nc.tensor.matmul(
                            ps[:], wt[:, 128 * di:128 * (di + 1)],
                            xt[:, 1024 * di + 512 * half:
                               1024 * di + 512 * (half + 1)],
                            start=(n == 0), stop=(n == 23))
                        n += 1
                nc.scalar.copy(qpT[:, sl], ps[:])

            # ---- gathered loads ----
            nc.sync.dma_start(
                kpT[:].bitcast(i32).rearrange("p (r x) -> p r x", r=8),
                agg_out[:, :, :1024].rearrange("r p x -> p r x"))
            vp_pk = projsb.tile([128, S // 2], i32, tag="vp_pk",
                                name="vp_pk")
            nc.sync.dma_start(
                vp_pk[:].rearrange("p (r x) -> p r x", r=8),
                agg_out[:, :, 1024:].rearrange("r p x -> p r x"))
            for i8 in range(8):
                nc.vector.tensor_copy(vp_sb[:, 1024 * i8:1024 * (i8 + 1)],
                                      vp_pk[:, 512 * i8:512 * (i8 + 1)]
                                      .bitcast(f16))

        # ---------- phase C: attention, 4 bands ----------
        with tc.tile_pool(name="statps", bufs=2, space="PSUM") as statps, \
             tc.tile_pool(name="sTps", bufs=2, space="PSUM") as sTps, \
             tc.tile_pool(name="oTps", bufs=1, space="PSUM") as oTps, \
             tc.tile_pool(name="smps", bufs=1, space="PSUM") as smps, \
             tc.tile_pool(name="tps", bufs=2, space="PSUM") as tps, \
             tc.tile_pool(name="mpool", bufs=2) as mpool, \
             tc.tile_pool(name="wpark", bufs=2) as wpark, \
             tc.tile_pool(name="pTpool", bufs=2) as pTpool, \
             tc.tile_pool(name="osb", bufs=2) as osb:

            for so in range(NBAND):
                rsl = slice(256 * so, 256 * (so + 1))   # qpT columns
                par = so % 2
                nbody = 16 * so                         # body tiles
                ntot = 16 * (so + 1)

                # ---- body stats: rows x [0, 2048*so) keys, unmasked ----
                m_rb = [mpool.tile([128, 1], f32, tag=f"m{h}", name=f"m{h}")
                        for h in range(2)]
                for h in range(2):
                    qsl = slice(256 * so + 128 * h, 256 * so + 128 * (h + 1))
                    for g in range(4 * so):
                        ps_s = statps.tile([128, 512], f32, tag="stat")
                        nc.tensor.matmul(ps_s[:], qpT[:, qsl],
                                         kpT[:, 512 * g:512 * (g + 1)],
                                         start=True, stop=True)
                        mx = mpool.tile([128, 1], f32, tag="mx")
                        nc.vector.reduce_max(mx[:], ps_s[:], axis=mybir.AxisListType.X)
                        if g == 0:
                            nc.vector.tensor_copy(m_rb[h][:], mx[:])
                        else:
                            nc.vector.tensor_max(m_rb[h][:], m_rb[h][:], mx[:])

                # ---- window QK: 8 tile-pairs, masked, parked in one tile ----
                wbig = wpark.tile([128, 4096], f32, tag="wpark", name="wbig")
                for i in range(8):
                    t = 16 * so + 2 * i
                    ps_w = sTps.tile([128, 512], f32, tag="sTw")
                    nc.tensor.matmul(ps_w[:, :256], kpT[:, 128 * t:128 * (t + 1)],
                                     qpT[:, rsl], start=True, stop=False,
                                     skip_group_check=True)
                    nc.tensor.matmul(ps_w[:, 256:],
                                     kpT[:, 128 * (t + 1):128 * (t + 2)],
                                     qpT[:, rsl], start=False, stop=True,
                                     skip_group_check=True)
                    msl = slice((par * 16 + 2 * i) * 256,
                                (par * 16 + 2 * i + 2) * 256)
                    nc.vector.tensor_add(wbig[:, 512 * i:512 * (i + 1)],
                                         ps_w[:], wmask[:, msl])

                # ---- window max: wide tree, transpose, reduce ----
                wtA = mpool.tile([128, 2048], f32, tag="wtA", name="wtA")
                for i in range(4):
                    nc.vector.tensor_max(wtA[:, 512 * i:512 * (i + 1)],
                                         wbig[:, 1024 * i:1024 * i + 512],
                                         wbig[:, 1024 * i + 512:1024 * (i + 1)])
                nc.vector.tensor_max(wtA[:, :512], wtA[:, :512], wtA[:, 512:1024])
                nc.vector.tensor_max(wtA[:, 1024:1536], wtA[:, 1024:1536],
                                     wtA[:, 1536:2048])
                nc.vector.tensor_max(wtA[:, :512], wtA[:, :512], wtA[:, 1024:1536])
                wmax = mpool.tile([128, 256], f32, tag="wmax", name="wmax")
                nc.vector.tensor_max(wmax[:], wtA[:, :256], wtA[:, 256:512])
                for h in range(2):
                    tp = tps.tile([128, 128], f32, tag="tp")
                    nc.tensor.transpose(tp[:], wmax[:, 128 * h:128 * (h + 1)],
                                        ident[:])
                    mx = mpool.tile([128, 1], f32, tag="mx2")
                    nc.vector.reduce_max(mx[:], tp[:], axis=mybir.AxisListType.X)
                    del tp
                    if so == 0:
                        nc.vector.tensor_copy(m_rb[h][:], mx[:])
                    else:
                        nc.vector.tensor_max(m_rb[h][:], m_rb[h][:], mx[:])

                # ---- mrep [1, 256] f32 (= m), mb_ps [128,256] (= -m bcast) ----
                mrep = mpool.tile([1, 256], f32r, tag="mrep", name="mrep")
                for h in range(2):
                    mp = tps.tile([128, 128], f32, tag="tp")
                    nc.tensor.transpose(mp[:1, :], m_rb[h][:], ident[:])
                    nc.scalar.copy(mrep[:, 128 * h:128 * (h + 1)], mp[:1, :])
                mb_ps = sTps.tile([128, 512], f32, tag="sTw")
                nc.tensor.matmul(mb_ps[:, :256], negones[:], mrep[:],
                                 start=True, stop=False, skip_group_check=True)
                nc.tensor.matmul(mb_ps[:, 256:], negones[:], mrep[:],
                                 start=False, stop=True, skip_group_check=True)
                mb_sb = mpool.tile([128, 512], f32, tag="mbsb", name="mbsb")
                nc.scalar.copy(mb_sb[:], mb_ps[:])

                # ---- main pass ----
                oT = oTps.tile([128, 256], f32, tag="oT")
                sm_t = smps.tile([128, 4], f32, tag="sm", name="sm")
                sm = [sm_t[:, 2 * h:2 * h + 2] for h in range(2)]

                def av(pt, t, first, last):
                    nc.tensor.matmul(oT[:], vp_sb[:, 128 * t:128 * (t + 1)],
                                     pt[:], start=first, stop=last)
                    # both halves' rowsums form ONE accumulation group in one
                    # PSUM bank (a group's first matmul zeroes the whole bank)
                    nc.tensor.matmul(sm[0], pt[:, 0:128], ones_col[:],
                                     start=first, stop=False,
                                     skip_group_check=True)
                    nc.tensor.matmul(sm[1], pt[:, 128:256], ones_col[:],
                                     start=False, stop=last,
                                     skip_group_check=True)

                # window tiles: add -m (wide), exp (wide), AV per tile
                ptb = pTpool.tile([128, 4096], f16, tag="pTbig", name="pTbig")
                for i in range(8):
                    wsl = slice(512 * i, 512 * (i + 1))
                    nc.vector.tensor_add(wbig[:, wsl], wbig[:, wsl], mb_sb[:])
                    nc.scalar.activation(ptb[:, wsl], wbig[:, wsl],
                                         mybir.ActivationFunctionType.Exp,
                                         scale=INV_SQRT_DK)
                for t_ in range(16):
                    t = 16 * so + t_
                    av(ptb[:, 256 * t_:256 * (t_ + 1)], t, t_ == 0,
                       (t_ == 15) and (nbody == 0))

                # body tiles: pairs per PSUM bank, rank-1 -m inline, wide exp
                for t0 in range(0, nbody, 2):
                    ps_b = sTps.tile([128, 512], f32, tag="sTw")
                    nc.tensor.matmul(ps_b[:, :256],
                                     kpT[:, 128 * t0:128 * (t0 + 1)],
                                     qpT[:, rsl], start=True, stop=False)
                    nc.tensor.matmul(ps_b[:, :256], negones[:], mrep[:],
                                     start=False, stop=False,
                                     skip_group_check=True)
                    nc.tensor.matmul(ps_b[:, 256:],
                                     kpT[:, 128 * (t0 + 1):128 * (t0 + 2)],
                                     qpT[:, rsl], start=False, stop=False,
                                     skip_group_check=True)
                    nc.tensor.matmul(ps_b[:, 256:], negones[:], mrep[:],
                                     start=False, stop=True,
                                     skip_group_check=True)
                    pt = pTpool.tile([128, 512], f16, tag="pT")
                    nc.scalar.activation(pt[:], ps_b[:],
                                         mybir.ActivationFunctionType.Exp,
                                         scale=INV_SQRT_DK)
                    av(pt[:, :256], t0, False, False)
                    av(pt[:, 256:], t0 + 1, False, t0 + 1 == nbody - 1)

                # ---- normalize + output projection ----
                oT_sb = osb.tile([128, 256], f32r, tag="oTsb")
                nc.vector.tensor_copy(oT_sb[:], oT[:])
                for h in range(2):
                    rsum = mpool.tile([128, 1], f32, tag="rsum")
                    nc.vector.reciprocal(rsum[:], sm[h][:, 0:1])
                    for dh in range(2):
                        ps_o = statps.tile([128, 512], f32, tag="stat")
                        nc.tensor.matmul(ps_o[:],
                                         oT_sb[:, 128 * h:128 * (h + 1)],
                                         wo_sb[:, 512 * dh:512 * (dh + 1)],
                                         start=True, stop=True)
                        oq = osb.tile([128, 512], f32, tag="oq")
                        nc.scalar.activation(
                            oq[:], ps_o[:], mybir.ActivationFunctionType.Copy,
                            scale=rsum[:])
                        nc.sync.dma_start(
                            o_sh[256 * so + 128 * h:256 * so + 128 * (h + 1),
                                 512 * dh:512 * (dh + 1)],
                            oq[:])

    nc.compile()
    return nc


def _host_inputs(q, k, v, w_q, w_k, w_v, w_o):
    f16 = np.float16

    def split(w):
        h = w.astype(f16)
        l = (w - h.astype(np.float32)).astype(f16)
        return h, l
    wq_h, wq_l = split(w_q)
    wk_h, wk_l = split(w_k)
    wv_h, wv_l = split(w_v)

    ident = np.eye(128, dtype=np.float32)
    negones = np.full((1, 128), -1.0, dtype=np.float32)
    ones_col = np.ones((128, 2), dtype=f16)

    # W mask strips: for parity slot par, pos = c if par==0 else 7-c:
    # strip[t_][kk, rr] = 0 if 128*t_ + kk <= 256*pos + rr else NEG_BIG
    kk = np.arange(128)[:, None]
    rr = np.arange(256)[None, :]

    in_maps = []
    for c in range(N_CORES):
        sbs = _sb_list(c)
        q_rows = np.concatenate([q[256 * m:256 * (m + 1)] for m in sbs])
        qT = np.ascontiguousarray(q_rows.T)
        qT_h, qT_l = split(qT)
        kT = np.ascontiguousarray(k[1024 * c:1024 * (c + 1)].T)
        kT_h, kT_l = split(kT)
        vT = np.ascontiguousarray(v[1024 * c:1024 * (c + 1)].T)
        vT_h = vT.astype(f16)

        wmask = np.empty((128, 2 * 16 * 256), np.float32)
        for par, pos in enumerate((c, 7 - c)):
            for t_ in range(16):
                strip = np.where(128 * t_ + kk <= 256 * pos + rr,
                                 0.0, NEG_BIG).astype(np.float32)
                wmask[:, (par * 16 + t_) * 256:(par * 16 + t_ + 1) * 256] = strip

        in_maps.append({
            "qT_h": qT_h, "qT_l": qT_l, "kT_h": kT_h, "kT_l": kT_l,
            "vT_h": vT_h,
            "wq_h": wq_h, "wq_l": wq_l, "wk_h": wk_h, "wk_l": wk_l,
            "wv_h": wv_h, "wv_l": wv_l, "wo": w_o,
            "wmask": wmask, "ident": ident, "negones": negones,
            "ones_col": ones_col,
        })
    return in_maps


def kernel(q, k, v, w_q, w_k, w_v, w_o):
    from concourse.bass_utils import run_bass_kernel_spmd

    q = np.asarray(q, dtype=np.float32)
    k = np.asarray(k, dtype=np.float32)
    v = np.asarray(v, dtype=np.float32)
    w_q = np.asarray(w_q, dtype=np.float32)
    w_k = np.asarray(w_k, dtype=np.float32)
    w_v = np.asarray(w_v, dtype=np.float32)
    w_o = np.asarray(w_o, dtype=np.float32)

    if "nc" not in _CACHE:
        _CACHE["nc"] = _build()
    nc = _CACHE["nc"]

    in_maps = _host_inputs(q, k, v, w_q, w_k, w_v, w_o)
    res = run_bass_kernel_spmd(nc, in_maps, list(range(N_CORES)))

    out = np.empty((S, D), dtype=np.float32)
    for c in range(N_CORES):
        o_sh = res.results[c]["o_sh"]
        for j, m in enumerate(_sb_list(c)):
            out[256 * m:256 * (m + 1)] = o_sh[256 * j:256 * (j + 1)]
    return out

